# revision 6
# baseline (speedup 1.0000x reference)
"""MoE grouped linear (DMoELinear) on 8 Trainium2 NeuronCores.

Expert-parallel sharding: tokens are sorted by expert id, so expert e's
tokens form one contiguous slice. Core e receives expert e's tokens
(padded to a uniform capacity C = max group size, so all cores run one
SPMD NEFF), expert e's weight and bias, and computes
    yT_e = (x_e @ W_e.T).T.bf16 + b_e.bf16
with the weight block as the stationary matmul operand and tokens as
the moving free dim (so C needs no 128-alignment). The weight is laid
out db-major ([16 output blocks] x [8 k-tiles] x [128k x 128dout]) so
DMA arrival order matches PE consumption order. The bias add fuses into
the PSUM-evacuation op as a per-partition scalar. The host does all
routing/gather in numpy.

v2 notes (trace-driven):
- x tiles DMA as full 2164B rows (one DMA per k-tile, rings alternating)
  instead of half-rows: ~2x ring throughput, so the input trickle phase
  shortens and the PE never starves on x.
- w0 leads ring A, w1 follows x0 on ring B; w2/w3 land right after the
  x stream (they are needed when the db0/db1 interleave finishes).
- PSUM evacuation runs on the Vector engine (DVE) for all but the last
  two output blocks: the Scalar engine's queue is busy issuing ring-B
  DMA descriptors early on, and ACTIVATE ops queued behind those issues
  stalled the PSUM recycle in the baseline.
- 18 warmup matmuls (not 44): enough to cover DMA lead-in and flip the
  PE HAM clock gate, without queuing real MMs behind dead work.
"""

import numpy as np
import ml_dtypes

N_TOK, D_IN, D_OUT, N_EXP = 8192, 1024, 2048, 8
N_CORES = 8
P = 128
NFREE = 512  # max matmul moving free dim (one PSUM bank of f32)

BF16 = ml_dtypes.bfloat16

_nc_cache: dict[int, object] = {}


def _chunks(C):
    out = []
    off = 0
    while off < C:
        cw = min(NFREE, C - off)
        out.append((off, cw))
        off += cw
    return out


def _build_bass(C: int):
    """Emit the per-core Bass/Tile kernel for token capacity C."""
    import concourse.bass as bass  # noqa: F401  (registers engines)
    import concourse.mybir as mybir
    import concourse.tile as tile
    from concourse import bacc

    dt = mybir.dt
    KT = D_IN // P      # 8 contraction tiles
    DB = D_OUT // P     # 16 output-row blocks
    chunks = _chunks(C)
    # The very last output block's post-matmul chain (evacuate -> DMA ->
    # semaphore) is serial with the end of the kernel: split it into four
    # even ~C/4 chunks so each link of that chain is short.
    if C <= 4 * NFREE:
        q, r = divmod(C, 4)
        fine = []
        off = 0
        for i in range(4):
            cw = q + (1 if i < r else 0)
            if cw:
                fine.append((off, cw))
            off += cw
        last_chunks = fine
    else:
        last_chunks = chunks

    nc = bacc.Bacc("TRN2", target_bir_lowering=False)

    # x: [ki][128k x C], one full-row DMA per k-tile.
    xT_d = nc.dram_tensor("xT", [D_IN, C], dt.bfloat16, kind="ExternalInput")
    # w: db-major, partition-contiguous: row (db*128+p) holds all KT
    # 128-wide k-slices for output block db, SBUF partition p.
    w_d = nc.dram_tensor("wdb", [DB * P, KT * P], dt.bfloat16, kind="ExternalInput")
    bias_d = nc.dram_tensor("biasp", [P, DB], dt.float32, kind="ExternalInput")
    y_d = nc.dram_tensor("yT", [D_OUT, C], dt.bfloat16, kind="ExternalOutput")

    with tile.TileContext(nc) as tc:
        with (
            tc.tile_pool(name="persist", bufs=1) as ppool,
            tc.tile_pool(name="yout", bufs=3) as ypool,
            tc.tile_pool(name="psum", bufs=8, space="PSUM") as pspool,
        ):
            w_tiles = [
                ppool.tile([P, KT * P], dt.bfloat16, name=f"w{db}", tag=f"w{db}")
                for db in range(DB)
            ]
            x_tiles = [
                ppool.tile([P, C], dt.bfloat16, name=f"x{ki}", tag=f"x{ki}")
                for ki in range(KT)
            ]
            bt = ppool.tile([P, DB], dt.float32, name="bias", tag="bias")

            def w_dma(eng, db):
                eng.dma_start(w_tiles[db][:], w_d[db * P:(db + 1) * P, :])

            def x_dma(eng, ki):
                eng.dma_start(x_tiles[ki][:], xT_d[ki * P:(ki + 1) * P, :])

            # Warm-tile memset on GpSimd (free right after the framework's
            # const-ap memsets) so the warmup matmuls can begin the moment
            # the Tensor engine clears its preamble.
            warm = ppool.tile([P, P], dt.bfloat16, name="warm", tag="warm")
            nc.gpsimd.memset(warm[:], 0.0)

            # Ring A (sync): x0h0, w0, x1, x3, x5, x7, w2, bias, evens.
            # Ring B (scalar): x0h1, w1, x2, x4, x6, w3, odds.
            # Per-ring FIFO order == PE consumption order; x rows are full
            # 2164B descriptors so the rings run near peak descriptor rate.
            # x0 is split across both rings (the only half-row DMAs) so the
            # first matmul's operand lands as early as possible.
            half = C // 2
            nc.sync.dma_start(x_tiles[0][:, :half], xT_d[:P, :half])
            nc.scalar.dma_start(x_tiles[0][:, half:], xT_d[:P, half:])
            w_dma(nc.sync, 0)
            w_dma(nc.scalar, 1)
            x_dma(nc.sync, 1)
            x_dma(nc.scalar, 2)
            x_dma(nc.sync, 3)
            x_dma(nc.scalar, 4)
            x_dma(nc.sync, 5)
            x_dma(nc.scalar, 6)
            x_dma(nc.sync, 7)
            w_dma(nc.sync, 2)
            w_dma(nc.scalar, 3)
            nc.sync.dma_start(bt[:], bias_d[:])
            for db in range(4, DB, 2):
                w_dma(nc.sync, db)
            for db in range(5, DB, 2):
                w_dma(nc.scalar, db)

            # Warm the PE's HAM clock gate with dummy matmuls on a zeroed
            # scratch tile while the input DMAs stream in (~3.4us of PE
            # activity flips the clock gate from 1.2GHz to 2.4GHz). 24 of
            # them bridge from the end of the Tensor preamble (~7.5us) to
            # x0-ready (~11us) so the real stream starts warm and gapless.
            wps = pspool.tile([P, P], dt.float32, name="wps", tag="ps")
            for _ in range(24):
                nc.tensor.matmul(wps[:], warm[:], warm[:], start=True, stop=True)

            all_psums = {}

            def chunks_for(db):
                return last_chunks if db == DB - 1 else chunks

            def alloc_psums(db):
                all_psums[db] = [
                    pspool.tile([P, NFREE], dt.float32, name=f"ps{db}_{j}", tag="ps")
                    for j in range(len(chunks_for(db)))
                ]

            def emit_mms(db, ki):
                lhsT = w_tiles[db][:, ki * P:(ki + 1) * P]
                for j, (off, cw) in enumerate(chunks_for(db)):
                    nc.tensor.matmul(
                        all_psums[db][j][:, :cw],
                        lhsT,
                        x_tiles[ki][:, off:off + cw],
                        start=(ki == 0),
                        stop=(ki == KT - 1),
                    )

            # First two db blocks interleaved by k-step: during the x DMA
            # trickle the PE consumes each freshly-landed x tile twice,
            # staying busier than a single db's worth of work would.
            alloc_psums(0)
            alloc_psums(1)
            for ki in range(KT):
                emit_mms(0, ki)
                emit_mms(1, ki)

            for db in range(DB):
                if db >= 2:
                    alloc_psums(db)
                    for ki in range(KT):
                        emit_mms(db, ki)
                psums = all_psums[db]
                ysb = ypool.tile([P, C], dt.bfloat16, name="ysb", tag="ysb")
                bias_col = bt[:, db:db + 1]
                last = db >= DB - 2
                for j, (off, cw) in enumerate(chunks_for(db)):
                    # fused PSUM->bf16 cast + per-partition bias add. DVE
                    # owns the steady-state evacuation (the Scalar queue is
                    # busy issuing ring-B DMAs early on); the two tail
                    # blocks alternate ACT/DVE so their chunks evacuate in
                    # parallel.
                    if last and j % 2 == 0:
                        nc.scalar.add(ysb[:, off:off + cw], psums[j][:, :cw], bias_col)
                    else:
                        nc.vector.tensor_scalar_add(
                            ysb[:, off:off + cw], psums[j][:, :cw], bias_col
                        )
                if last:
                    # tail blocks: per-chunk DMAs on alternating rings so the
                    # final transfer lands ASAP
                    for j, (off, cw) in enumerate(chunks_for(db)):
                        eng = nc.sync if (db + j) % 2 == 0 else nc.scalar
                        eng.dma_start(
                            y_d[db * P:(db + 1) * P, off:off + cw],
                            ysb[:, off:off + cw],
                        )
                else:
                    eng = nc.sync if db % 2 == 0 else nc.scalar
                    eng.dma_start(y_d[db * P:(db + 1) * P, :], ysb[:])

    nc.compile()
    return nc


def _run_spmd(in_maps, C, trace=False, trace_cores=None):
    from concourse.bass_utils import run_bass_kernel_spmd

    nc = _nc_cache.get(C)
    if nc is None:
        nc = _build_bass(C)
        _nc_cache[C] = nc
    return run_bass_kernel_spmd(
        nc,
        in_maps,
        core_ids=list(range(N_CORES)),
        trace=trace,
        trace_cores=trace_cores,
    )


def _prepare(x, weight, bias, ids_sorted):
    """Host-side routing: returns (in_maps, C, counts, starts)."""
    x = np.asarray(x)
    weight = np.asarray(weight)
    bias = np.asarray(bias)
    ids = np.asarray(ids_sorted)

    counts = np.bincount(ids, minlength=N_EXP).astype(np.int64)
    starts = np.zeros(N_EXP, dtype=np.int64)
    starts[1:] = np.cumsum(counts)[:-1]
    C = max(int(counts.max()), 2)
    C += C % 2

    xb = x.astype(BF16)
    in_maps = []
    for e in range(N_EXP):
        n_e = int(counts[e])
        xeT = np.zeros((D_IN, C), dtype=BF16)
        if n_e:
            xeT[:, :n_e] = xb[starts[e]:starts[e] + n_e].T
        # db-major weight: row (db*128+p) = [w.T[kt*128+p, db*128+d] for kt, d]
        weT = weight[e].T.astype(BF16)  # [d_in, d_out]
        wdb = np.ascontiguousarray(
            weT.reshape(D_IN // P, P, D_OUT // P, P).transpose(2, 1, 0, 3)
        ).reshape(D_OUT, D_IN)
        bp = np.ascontiguousarray(
            bias[e].astype(BF16).astype(np.float32).reshape(D_OUT // P, P).T
        )
        in_maps.append({"xT": xeT, "wdb": wdb, "biasp": bp})
    return in_maps, C, counts, starts


def _assemble(results, counts, starts):
    out = np.empty((N_TOK, D_OUT), dtype=BF16)
    for e in range(N_EXP):
        n_e = int(counts[e])
        if n_e:
            out[starts[e]:starts[e] + n_e] = results[e]["yT"][:, :n_e].T
    return out


def kernel(x, weight, bias, ids_sorted):
    in_maps, C, counts, starts = _prepare(x, weight, bias, ids_sorted)
    res = _run_spmd(in_maps, C)
    return _assemble(res.results, counts, starts)


# revision 9
# speedup vs baseline: 1.0035x; 1.0035x over previous
"""MoE grouped linear (DMoELinear) on 8 Trainium2 NeuronCores.

Expert-parallel sharding: tokens are sorted by expert id, so expert e's
tokens form one contiguous slice. Core e receives expert e's tokens
(padded to a uniform capacity C = max group size, so all cores run one
SPMD NEFF), expert e's weight and bias, and computes
    yT_e = (x_e @ W_e.T).T.bf16 + b_e.bf16
with the weight block as the stationary matmul operand and tokens as
the moving free dim (so C needs no 128-alignment). The weight is laid
out db-major ([16 output blocks] x [8 k-tiles] x [128k x 128dout]) so
DMA arrival order matches PE consumption order. The bias add fuses into
the PSUM-evacuation op as a per-partition scalar. The host does all
routing/gather in numpy.

v2 notes (trace-driven):
- x tiles DMA as full 2164B rows (one DMA per k-tile, rings alternating)
  instead of half-rows: ~2x ring throughput, so the input trickle phase
  shortens and the PE never starves on x.
- w0 leads ring A, w1 follows x0 on ring B; w2/w3 land right after the
  x stream (they are needed when the db0/db1 interleave finishes).
- PSUM evacuation runs on the Vector engine (DVE) for all but the last
  two output blocks: the Scalar engine's queue is busy issuing ring-B
  DMA descriptors early on, and ACTIVATE ops queued behind those issues
  stalled the PSUM recycle in the baseline.
- 18 warmup matmuls (not 44): enough to cover DMA lead-in and flip the
  PE HAM clock gate, without queuing real MMs behind dead work.
"""

import numpy as np
import ml_dtypes

N_TOK, D_IN, D_OUT, N_EXP = 8192, 1024, 2048, 8
N_CORES = 8
P = 128
NFREE = 512  # max matmul moving free dim (one PSUM bank of f32)

BF16 = ml_dtypes.bfloat16

_nc_cache: dict[int, object] = {}


def _chunks(C):
    out = []
    off = 0
    while off < C:
        cw = min(NFREE, C - off)
        out.append((off, cw))
        off += cw
    return out


def _build_bass(C: int):
    """Emit the per-core Bass/Tile kernel for token capacity C."""
    import concourse.bass as bass  # noqa: F401  (registers engines)
    import concourse.mybir as mybir
    import concourse.tile as tile
    from concourse import bacc

    dt = mybir.dt
    KT = D_IN // P      # 8 contraction tiles
    DB = D_OUT // P     # 16 output-row blocks
    chunks = _chunks(C)
    last_chunks = chunks

    nc = bacc.Bacc("TRN2", target_bir_lowering=False)

    # x: [ki][128k x C], one full-row DMA per k-tile.
    xT_d = nc.dram_tensor("xT", [D_IN, C], dt.bfloat16, kind="ExternalInput")
    # w: db-major, partition-contiguous: row (db*128+p) holds all KT
    # 128-wide k-slices for output block db, SBUF partition p.
    w_d = nc.dram_tensor("wdb", [DB * P, KT * P], dt.bfloat16, kind="ExternalInput")
    bias_d = nc.dram_tensor("biasp", [P, DB], dt.float32, kind="ExternalInput")
    y_d = nc.dram_tensor("yT", [D_OUT, C], dt.bfloat16, kind="ExternalOutput")

    with tile.TileContext(nc) as tc:
        with (
            tc.tile_pool(name="persist", bufs=1) as ppool,
            tc.tile_pool(name="yout", bufs=3) as ypool,
            tc.tile_pool(name="psum", bufs=8, space="PSUM") as pspool,
        ):
            w_tiles = [
                ppool.tile([P, KT * P], dt.bfloat16, name=f"w{db}", tag=f"w{db}")
                for db in range(DB)
            ]
            x_tiles = [
                ppool.tile([P, C], dt.bfloat16, name=f"x{ki}", tag=f"x{ki}")
                for ki in range(KT)
            ]
            bt = ppool.tile([P, DB], dt.float32, name="bias", tag="bias")

            def w_dma(eng, db):
                eng.dma_start(w_tiles[db][:], w_d[db * P:(db + 1) * P, :])

            def x_dma(eng, ki):
                eng.dma_start(x_tiles[ki][:], xT_d[ki * P:(ki + 1) * P, :])

            # Warm-tile memset on GpSimd (free right after the framework's
            # const-ap memsets) so the warmup matmuls can begin the moment
            # the Tensor engine clears its preamble.
            warm = ppool.tile([P, P], dt.bfloat16, name="warm", tag="warm")
            nc.gpsimd.memset(warm[:], 0.0)

            # Ring A (sync): w0, x1, x3, x5, x7, w2, bias, evens.
            # Ring B (scalar): x0, w1, x2, x4, x6, w3, odds.
            # Per-ring FIFO order == PE consumption order; x rows are full
            # 2164B descriptors so the 16 shared SDMA engines run near
            # peak descriptor rate (~360 GB/s aggregate).
            w_dma(nc.sync, 0)
            x_dma(nc.scalar, 0)
            w_dma(nc.scalar, 1)
            x_dma(nc.sync, 1)
            x_dma(nc.scalar, 2)
            x_dma(nc.sync, 3)
            x_dma(nc.scalar, 4)
            x_dma(nc.sync, 5)
            x_dma(nc.scalar, 6)
            x_dma(nc.sync, 7)
            w_dma(nc.sync, 2)
            w_dma(nc.scalar, 3)
            nc.sync.dma_start(bt[:], bias_d[:])
            for db in range(4, DB, 2):
                w_dma(nc.sync, db)
            for db in range(5, DB, 2):
                w_dma(nc.scalar, db)

            # Warm the PE's HAM clock gate with dummy matmuls on a zeroed
            # scratch tile while the input DMAs stream in (~3.4us of PE
            # activity flips the clock gate from 1.2GHz to 2.4GHz). 26
            # bridge from the end of the Tensor preamble (~7.5us) to
            # x0-ready (~11.2us); a few more are interleaved between the
            # first real k-steps below to pad the x1/x2 arrival gaps so the
            # clock gate never drops back to 1.2GHz.
            wps = pspool.tile([P, P], dt.float32, name="wps", tag="ps")

            def warmup(n):
                for _ in range(n):
                    nc.tensor.matmul(wps[:], warm[:], warm[:], start=True, stop=True)

            warmup(26)

            all_psums = {}

            def chunks_for(db):
                return last_chunks if db == DB - 1 else chunks

            def alloc_psums(db):
                all_psums[db] = [
                    pspool.tile([P, NFREE], dt.float32, name=f"ps{db}_{j}", tag="ps")
                    for j in range(len(chunks_for(db)))
                ]

            def emit_mms(db, ki):
                lhsT = w_tiles[db][:, ki * P:(ki + 1) * P]
                for j, (off, cw) in enumerate(chunks_for(db)):
                    nc.tensor.matmul(
                        all_psums[db][j][:, :cw],
                        lhsT,
                        x_tiles[ki][:, off:off + cw],
                        start=(ki == 0),
                        stop=(ki == KT - 1),
                    )

            # First two db blocks interleaved by k-step: during the x DMA
            # trickle the PE consumes each freshly-landed x tile twice,
            # staying busier than a single db's worth of work would. The
            # first two k-steps outrun the rings; elastic warmup padding
            # keeps the PE busy (and the clock gate hot) while x1/x2 land.
            alloc_psums(0)
            alloc_psums(1)
            for ki in range(KT):
                emit_mms(0, ki)
                emit_mms(1, ki)
                if ki == 0:
                    warmup(8)
                elif ki == 1:
                    warmup(5)

            for db in range(DB):
                if db >= 2:
                    alloc_psums(db)
                    for ki in range(KT):
                        emit_mms(db, ki)
                psums = all_psums[db]
                ysb = ypool.tile([P, C], dt.bfloat16, name="ysb", tag="ysb")
                bias_col = bt[:, db:db + 1]
                last = db >= DB - 2
                for j, (off, cw) in enumerate(chunks_for(db)):
                    # fused PSUM->bf16 cast + per-partition bias add. DVE
                    # owns the steady-state evacuation (the Scalar queue is
                    # busy issuing ring-B DMAs early on); the two tail
                    # blocks alternate ACT/DVE so their chunks evacuate in
                    # parallel.
                    if last and j % 2 == 0:
                        nc.scalar.add(ysb[:, off:off + cw], psums[j][:, :cw], bias_col)
                    else:
                        nc.vector.tensor_scalar_add(
                            ysb[:, off:off + cw], psums[j][:, :cw], bias_col
                        )
                if last:
                    # tail blocks: per-chunk DMAs on alternating rings so the
                    # final transfer lands ASAP
                    for j, (off, cw) in enumerate(chunks_for(db)):
                        eng = nc.sync if (db + j) % 2 == 0 else nc.scalar
                        eng.dma_start(
                            y_d[db * P:(db + 1) * P, off:off + cw],
                            ysb[:, off:off + cw],
                        )
                else:
                    eng = nc.sync if db % 2 == 0 else nc.scalar
                    eng.dma_start(y_d[db * P:(db + 1) * P, :], ysb[:])

    nc.compile()
    return nc


def _run_spmd(in_maps, C, trace=False, trace_cores=None):
    from concourse.bass_utils import run_bass_kernel_spmd

    nc = _nc_cache.get(C)
    if nc is None:
        nc = _build_bass(C)
        _nc_cache[C] = nc
    return run_bass_kernel_spmd(
        nc,
        in_maps,
        core_ids=list(range(N_CORES)),
        trace=trace,
        trace_cores=trace_cores,
    )


def _prepare(x, weight, bias, ids_sorted):
    """Host-side routing: returns (in_maps, C, counts, starts)."""
    x = np.asarray(x)
    weight = np.asarray(weight)
    bias = np.asarray(bias)
    ids = np.asarray(ids_sorted)

    counts = np.bincount(ids, minlength=N_EXP).astype(np.int64)
    starts = np.zeros(N_EXP, dtype=np.int64)
    starts[1:] = np.cumsum(counts)[:-1]
    C = max(int(counts.max()), 2)
    C += C % 2

    xb = x.astype(BF16)
    in_maps = []
    for e in range(N_EXP):
        n_e = int(counts[e])
        xeT = np.zeros((D_IN, C), dtype=BF16)
        if n_e:
            xeT[:, :n_e] = xb[starts[e]:starts[e] + n_e].T
        # db-major weight: row (db*128+p) = [w.T[kt*128+p, db*128+d] for kt, d]
        weT = weight[e].T.astype(BF16)  # [d_in, d_out]
        wdb = np.ascontiguousarray(
            weT.reshape(D_IN // P, P, D_OUT // P, P).transpose(2, 1, 0, 3)
        ).reshape(D_OUT, D_IN)
        bp = np.ascontiguousarray(
            bias[e].astype(BF16).astype(np.float32).reshape(D_OUT // P, P).T
        )
        in_maps.append({"xT": xeT, "wdb": wdb, "biasp": bp})
    return in_maps, C, counts, starts


def _assemble(results, counts, starts):
    out = np.empty((N_TOK, D_OUT), dtype=BF16)
    for e in range(N_EXP):
        n_e = int(counts[e])
        if n_e:
            out[starts[e]:starts[e] + n_e] = results[e]["yT"][:, :n_e].T
    return out


def kernel(x, weight, bias, ids_sorted):
    in_maps, C, counts, starts = _prepare(x, weight, bias, ids_sorted)
    res = _run_spmd(in_maps, C)
    return _assemble(res.results, counts, starts)


# revision 11
# speedup vs baseline: 1.0209x; 1.0174x over previous
"""MoE grouped linear (DMoELinear) on 8 Trainium2 NeuronCores.

Expert-parallel sharding: tokens are sorted by expert id, so expert e's
tokens form one contiguous slice. Core e receives expert e's tokens
(padded to a uniform capacity C = max group size, so all cores run one
SPMD NEFF), expert e's weight and bias, and computes
    yT_e = (x_e @ W_e.T).T.bf16 + b_e.bf16
with the weight block as the stationary matmul operand and tokens as
the moving free dim (so C needs no 128-alignment). The weight is laid
out db-major ([16 output blocks] x [8 k-tiles] x [128k x 128dout]) so
DMA arrival order matches PE consumption order. The bias add fuses into
the PSUM-evacuation op as a per-partition scalar. The host does all
routing/gather in numpy.

v2 notes (trace-driven):
- x tiles DMA as full 2164B rows (one DMA per k-tile, rings alternating)
  instead of half-rows: ~2x ring throughput, so the input trickle phase
  shortens and the PE never starves on x.
- w0 leads ring A, w1 follows x0 on ring B; w2/w3 land right after the
  x stream (they are needed when the db0/db1 interleave finishes).
- PSUM evacuation runs on the Vector engine (DVE) for all but the last
  two output blocks: the Scalar engine's queue is busy issuing ring-B
  DMA descriptors early on, and ACTIVATE ops queued behind those issues
  stalled the PSUM recycle in the baseline.
- 18 warmup matmuls (not 44): enough to cover DMA lead-in and flip the
  PE HAM clock gate, without queuing real MMs behind dead work.
"""

import numpy as np
import ml_dtypes

N_TOK, D_IN, D_OUT, N_EXP = 8192, 1024, 2048, 8
N_CORES = 8
P = 128
NFREE = 512  # max matmul moving free dim (one PSUM bank of f32)

BF16 = ml_dtypes.bfloat16

_nc_cache: dict[int, object] = {}


def _chunks(C):
    out = []
    off = 0
    while off < C:
        cw = min(NFREE, C - off)
        out.append((off, cw))
        off += cw
    return out


def _build_bass(C: int):
    """Emit the per-core Bass/Tile kernel for token capacity C."""
    import concourse.bass as bass  # noqa: F401  (registers engines)
    import concourse.mybir as mybir
    import concourse.tile as tile
    from concourse import bacc

    dt = mybir.dt
    KT = D_IN // P      # 8 contraction tiles
    DB = D_OUT // P     # 16 output-row blocks
    chunks = _chunks(C)
    last_chunks = chunks

    nc = bacc.Bacc("TRN2", target_bir_lowering=False)

    # x: [ki][128k x C], one full-row DMA per k-tile.
    xT_d = nc.dram_tensor("xT", [D_IN, C], dt.bfloat16, kind="ExternalInput")
    # w: db-major, partition-contiguous: row (db*128+p) holds all KT
    # 128-wide k-slices for output block db, SBUF partition p.
    w_d = nc.dram_tensor("wdb", [DB * P, KT * P], dt.bfloat16, kind="ExternalInput")
    bias_d = nc.dram_tensor("biasp", [P, DB], dt.float32, kind="ExternalInput")
    y_d = nc.dram_tensor("yT", [D_OUT, C], dt.bfloat16, kind="ExternalOutput")

    with tile.TileContext(nc) as tc:
        with (
            tc.tile_pool(name="persist", bufs=1) as ppool,
            tc.tile_pool(name="yout", bufs=3) as ypool,
            tc.tile_pool(name="psum", bufs=8, space="PSUM") as pspool,
        ):
            w_tiles = [
                ppool.tile([P, KT * P], dt.bfloat16, name=f"w{db}", tag=f"w{db}")
                for db in range(DB)
            ]
            x_tiles = [
                ppool.tile([P, C], dt.bfloat16, name=f"x{ki}", tag=f"x{ki}")
                for ki in range(KT)
            ]
            bt = ppool.tile([P, DB], dt.float32, name="bias", tag="bias")

            def w_dma(eng, db):
                eng.dma_start(w_tiles[db][:], w_d[db * P:(db + 1) * P, :])

            def x_dma(eng, ki):
                eng.dma_start(x_tiles[ki][:], xT_d[ki * P:(ki + 1) * P, :])

            # Warm-tile memset on GpSimd (free right after the framework's
            # const-ap memsets) so the warmup matmuls can begin the moment
            # the Tensor engine clears its preamble.
            warm = ppool.tile([P, P], dt.bfloat16, name="warm", tag="warm")
            nc.gpsimd.memset(warm[:], 0.0)

            # Ring A (sync): w0, x1, x3, x5, x7, w2, bias, evens.
            # Ring B (scalar): x0, w1, x2, x4, x6, w3, odds.
            # Per-ring FIFO order == PE consumption order; x rows are full
            # 2164B descriptors so the 16 shared SDMA engines run near
            # peak descriptor rate (~360 GB/s aggregate).
            w_dma(nc.sync, 0)
            x_dma(nc.scalar, 0)
            w_dma(nc.scalar, 1)
            x_dma(nc.sync, 1)
            x_dma(nc.scalar, 2)
            x_dma(nc.sync, 3)
            x_dma(nc.scalar, 4)
            x_dma(nc.sync, 5)
            x_dma(nc.scalar, 6)
            x_dma(nc.sync, 7)
            w_dma(nc.sync, 2)
            w_dma(nc.scalar, 3)
            nc.sync.dma_start(bt[:], bias_d[:])
            for db in range(4, DB, 2):
                w_dma(nc.sync, db)
            for db in range(5, DB, 2):
                w_dma(nc.scalar, db)

            # Warm the PE's HAM clock gate with dummy matmuls on a zeroed
            # scratch tile while the input DMAs stream in. The gate flips
            # from 1.2GHz to 2.4GHz only after ~3.4us of UNBROKEN PE
            # activity; any idle gap restarts that window. So the leading
            # block must run past the flip point on its own (38 cold MMs
            # from ~7.5us reach ~11.2us > 10.9us flip), and the first real
            # k-steps get padded below so the x1/x2 arrival gaps never idle
            # the PE. Overshooting pads is cheap (56ns/warm MM); a single
            # idle gap before the flip costs ~2.5us of half-rate matmuls.
            wps = pspool.tile([P, P], dt.float32, name="wps", tag="ps")

            def warmup(n):
                for _ in range(n):
                    nc.tensor.matmul(wps[:], warm[:], warm[:], start=True, stop=True)

            warmup(38)

            all_psums = {}

            def chunks_for(db):
                return last_chunks if db == DB - 1 else chunks

            def alloc_psums(db):
                all_psums[db] = [
                    pspool.tile([P, NFREE], dt.float32, name=f"ps{db}_{j}", tag="ps")
                    for j in range(len(chunks_for(db)))
                ]

            def emit_mms(db, ki):
                lhsT = w_tiles[db][:, ki * P:(ki + 1) * P]
                for j, (off, cw) in enumerate(chunks_for(db)):
                    nc.tensor.matmul(
                        all_psums[db][j][:, :cw],
                        lhsT,
                        x_tiles[ki][:, off:off + cw],
                        start=(ki == 0),
                        stop=(ki == KT - 1),
                    )

            # First two db blocks interleaved by k-step: during the x DMA
            # trickle the PE consumes each freshly-landed x tile twice,
            # staying busier than a single db's worth of work would. The
            # first two k-steps outrun the rings; elastic warmup padding
            # keeps the PE busy (and the clock gate hot) while x1/x2 land.
            alloc_psums(0)
            alloc_psums(1)
            for ki in range(KT):
                emit_mms(0, ki)
                emit_mms(1, ki)
                if ki == 0:
                    warmup(12)
                elif ki == 1:
                    warmup(3)

            for db in range(DB):
                if db >= 2:
                    alloc_psums(db)
                    for ki in range(KT):
                        emit_mms(db, ki)
                psums = all_psums[db]
                ysb = ypool.tile([P, C], dt.bfloat16, name="ysb", tag="ysb")
                bias_col = bt[:, db:db + 1]
                last = db >= DB - 2
                for j, (off, cw) in enumerate(chunks_for(db)):
                    # fused PSUM->bf16 cast + per-partition bias add. DVE
                    # owns the steady-state evacuation (the Scalar queue is
                    # busy issuing ring-B DMAs early on); the two tail
                    # blocks alternate ACT/DVE so their chunks evacuate in
                    # parallel.
                    if last and j % 2 == 0:
                        nc.scalar.add(ysb[:, off:off + cw], psums[j][:, :cw], bias_col)
                    else:
                        nc.vector.tensor_scalar_add(
                            ysb[:, off:off + cw], psums[j][:, :cw], bias_col
                        )
                if last:
                    # tail blocks: per-chunk DMAs on alternating rings so the
                    # final transfer lands ASAP
                    for j, (off, cw) in enumerate(chunks_for(db)):
                        eng = nc.sync if (db + j) % 2 == 0 else nc.scalar
                        eng.dma_start(
                            y_d[db * P:(db + 1) * P, off:off + cw],
                            ysb[:, off:off + cw],
                        )
                else:
                    eng = nc.sync if db % 2 == 0 else nc.scalar
                    eng.dma_start(y_d[db * P:(db + 1) * P, :], ysb[:])

    nc.compile()
    return nc


def _run_spmd(in_maps, C, trace=False, trace_cores=None):
    from concourse.bass_utils import run_bass_kernel_spmd

    nc = _nc_cache.get(C)
    if nc is None:
        nc = _build_bass(C)
        _nc_cache[C] = nc
    return run_bass_kernel_spmd(
        nc,
        in_maps,
        core_ids=list(range(N_CORES)),
        trace=trace,
        trace_cores=trace_cores,
    )


def _prepare(x, weight, bias, ids_sorted):
    """Host-side routing: returns (in_maps, C, counts, starts)."""
    x = np.asarray(x)
    weight = np.asarray(weight)
    bias = np.asarray(bias)
    ids = np.asarray(ids_sorted)

    counts = np.bincount(ids, minlength=N_EXP).astype(np.int64)
    starts = np.zeros(N_EXP, dtype=np.int64)
    starts[1:] = np.cumsum(counts)[:-1]
    C = max(int(counts.max()), 2)
    C += C % 2

    xb = x.astype(BF16)
    in_maps = []
    for e in range(N_EXP):
        n_e = int(counts[e])
        xeT = np.zeros((D_IN, C), dtype=BF16)
        if n_e:
            xeT[:, :n_e] = xb[starts[e]:starts[e] + n_e].T
        # db-major weight: row (db*128+p) = [w.T[kt*128+p, db*128+d] for kt, d]
        weT = weight[e].T.astype(BF16)  # [d_in, d_out]
        wdb = np.ascontiguousarray(
            weT.reshape(D_IN // P, P, D_OUT // P, P).transpose(2, 1, 0, 3)
        ).reshape(D_OUT, D_IN)
        bp = np.ascontiguousarray(
            bias[e].astype(BF16).astype(np.float32).reshape(D_OUT // P, P).T
        )
        in_maps.append({"xT": xeT, "wdb": wdb, "biasp": bp})
    return in_maps, C, counts, starts


def _assemble(results, counts, starts):
    out = np.empty((N_TOK, D_OUT), dtype=BF16)
    for e in range(N_EXP):
        n_e = int(counts[e])
        if n_e:
            out[starts[e]:starts[e] + n_e] = results[e]["yT"][:, :n_e].T
    return out


def kernel(x, weight, bias, ids_sorted):
    in_maps, C, counts, starts = _prepare(x, weight, bias, ids_sorted)
    res = _run_spmd(in_maps, C)
    return _assemble(res.results, counts, starts)
